# revision 8
# baseline (speedup 1.0000x reference)
"""Trainium2 Bass kernel for causal GQA attention (B=1, T=4096, D=2048,
H=16, Hkv=4, Dh=128, RoPE) sharded over 8 NeuronCores.

Sharding: tensor-parallel over heads — each core owns 2 q-heads and the
kv head they share (core c: q-heads {2c, 2c+1}, kv head c//2). Each core
computes its q/k/v projections, RoPE, causal attention and its partial
o_proj contribution y_c = O_c @ Wo_c; the host sums the 8 partials.

On-device dataflow (everything bf16 into the PE, f32 accumulation):
  xT tiles [c,t] -> Q^T/K^T/V^T [d,t] -> RoPE (DVE) -> S^T = K^T.T@Q^T
  per (j-tile 128, q-tile 512) -> exp on ACT (PSUM->SBUF bf16, fused
  1/sqrt(dh) scale) -> causal mask on diagonal blocks (DVE mul with
  precomputed mask) -> O_aug[q,129] += P^T.T @ [V | 1] (PE, PSUM
  accumulation; col 128 gives the softmax denominator) -> reciprocal +
  per-partition scale on ACT -> PE transpose -> O^T -> y = O^T.T @ Wo.
"""

import sys

sys.path.insert(0, "/opt/trn_rl_repo")

import math
from contextlib import ExitStack

import ml_dtypes
import numpy as np

import concourse.bass as bass
import concourse.tile as tile
from concourse import bacc, mybir
from concourse.bass_utils import run_bass_kernel_spmd
from concourse.masks import make_identity

BF16 = mybir.dt.bfloat16
F32 = mybir.dt.float32
NPBF16 = ml_dtypes.bfloat16

B, T, D = 1, 4096, 2048
H, HKV, DH = 16, 4, 128
GROUP = H // HKV
ROPE_BASE = 10000.0
N_CORES = 8
HL = H // N_CORES  # q-heads per core
KC = D // 128      # contraction tiles for projections
NQ = T // 512      # 512-wide q tiles
NJ = T // 128      # 128-wide kv tiles
NT = T // 128      # 128-row output tiles
NM = D // 512      # 512-wide output column tiles
SCALE = 1.0 / math.sqrt(DH)

Exp = mybir.ActivationFunctionType.Exp
Copy = mybir.ActivationFunctionType.Copy


def _build(nc):
    xp = nc.dram_tensor("xp", [NQ, 128, KC, 512], BF16, kind="ExternalInput").ap()
    wqkv = nc.dram_tensor("wqkv", [128, KC, 4, 128], BF16, kind="ExternalInput").ap()
    wo = nc.dram_tensor("wo", [128, HL, D], BF16, kind="ExternalInput").ap()
    cos2 = nc.dram_tensor("cos2", [128, T], BF16, kind="ExternalInput").ap()
    sinsig = nc.dram_tensor("sinsig", [128, T], BF16, kind="ExternalInput").ap()
    perm = nc.dram_tensor("perm", [128, 128], BF16, kind="ExternalInput").ap()
    maskd = nc.dram_tensor("maskd", [128, 4, 512], BF16, kind="ExternalInput").ap()
    y = nc.dram_tensor("y", [T, D], F32, kind="ExternalOutput").ap()

    with tile.TileContext(nc) as tc, ExitStack() as ctx:
        const = ctx.enter_context(tc.tile_pool(name="const", bufs=1))
        xpool = ctx.enter_context(tc.tile_pool(name="xp", bufs=2))
        psum = ctx.enter_context(tc.tile_pool(name="ps", bufs=2, space="PSUM"))
        opsum = ctx.enter_context(tc.tile_pool(name="ops", bufs=4, space="PSUM"))
        tpsum = ctx.enter_context(tc.tile_pool(name="tps", bufs=2, space="PSUM"))
        ppool = ctx.enter_context(tc.tile_pool(name="pt", bufs=3))
        spool = ctx.enter_context(tc.tile_pool(name="sm", bufs=4))
        yrow = ctx.enter_context(tc.tile_pool(name="yr", bufs=2))

        wqkv_sb = const.tile([128, KC, 4, 128], BF16, tag="wqkv")
        nc.sync.dma_start(wqkv_sb[:], wqkv[:])
        wo_sb = const.tile([128, HL, D], BF16, tag="wo")
        nc.sync.dma_start(wo_sb[:], wo[:])
        cos_sb = const.tile([128, T], BF16, tag="cos")
        nc.sync.dma_start(cos_sb[:], cos2[:])
        sin_sb = const.tile([128, T], BF16, tag="sin")
        nc.sync.dma_start(sin_sb[:], sinsig[:])
        perm_sb = const.tile([128, 128], BF16, tag="perm")
        nc.sync.dma_start(perm_sb[:], perm[:])
        mask_sb = const.tile([128, 4, 512], BF16, tag="mask")
        nc.sync.dma_start(mask_sb[:], maskd[:])
        ident = const.tile([128, 128], BF16, tag="ident")
        make_identity(nc, ident[:])

        qkvT = const.tile([128, 4, T], BF16, tag="qkvT")   # Q0,Q1,K,V (as [d,t]); RoPE in place
        vaug = const.tile([128, NJ, 129], BF16, tag="vaug")  # V natural + ones col
        oT = const.tile([128, HL, T], BF16, tag="oT")

        # Phase 1: fused q/k/v projection, outputs transposed [d, t].
        for n in range(NQ):
            xt = xpool.tile([128, KC, 512], BF16, tag="xt")
            nc.sync.dma_start(xt[:], xp[n])
            for m in range(4):
                ps = psum.tile([128, 512], F32, tag="ps")
                for k in range(KC):
                    nc.tensor.matmul(
                        ps[:],
                        lhsT=wqkv_sb[:, k, m, :],
                        rhs=xt[:, k, :],
                        start=(k == 0),
                        stop=(k == KC - 1),
                    )
                nc.vector.tensor_copy(qkvT[:, m, bass.ts(n, 512)], ps[:])

        # Phase 2: RoPE on q0, q1, k, in place. The rotate_half partition
        # swap is a permutation matmul on the PE (elementwise engines
        # cannot shift partitions); then dst = src*cos + swap(src)*[-sin;sin].
        for i in range(3):
            src = qkvT[:, i, :]
            swp = xpool.tile([128, T], BF16, tag="swp", bufs=2, name=f"swp{i}")
            for n in range(NQ):
                sw_ps = psum.tile([128, 512], F32, tag="ps", name=f"swps{i}_{n}")
                nc.tensor.matmul(
                    sw_ps[:],
                    lhsT=perm_sb[:],
                    rhs=src[:, bass.ts(n, 512)],
                    start=True,
                    stop=True,
                )
                nc.vector.tensor_copy(swp[:, bass.ts(n, 512)], sw_ps[:])
            nc.vector.tensor_mul(src, src, cos_sb[:])
            nc.vector.tensor_mul(swp[:], swp[:], sin_sb[:])
            nc.vector.tensor_add(src, src, swp[:])

        # Phase 3: V^T -> V natural [j, d] (PE transpose) + ones column.
        nc.vector.memset(vaug[:, :, 128], 1.0)
        for jt in range(NJ):
            tp = tpsum.tile([128, 128], BF16, tag="tp")
            nc.tensor.transpose(tp[:], qkvT[:, 3, bass.ts(jt, 128)], ident[:])
            nc.vector.tensor_copy(vaug[:, jt, 0:128], tp[:])

        # Phase 4: causal attention per head, streamed over kv tiles.
        kT = qkvT[:, 2, :]
        for h in range(HL):
            qT = qkvT[:, h, :]
            for qi in range(NQ):
                njt = 4 * (qi + 1)  # kv tiles below/at the diagonal
                oacc = [
                    opsum.tile([128, 129], F32, tag="oacc", name=f"oacc{h}_{qi}_{i}")
                    for i in range(4)
                ]
                for jt in range(njt):
                    sps = psum.tile([128, 512], F32, tag="ps")
                    nc.tensor.matmul(
                        sps[:],
                        lhsT=kT[:, bass.ts(jt, 128)],
                        rhs=qT[:, bass.ts(qi, 512)],
                        start=True,
                        stop=True,
                    )
                    pt = ppool.tile([128, 512], BF16, tag="pt")
                    nc.scalar.activation(pt[:], sps[:], Exp, scale=SCALE)
                    kd = jt - 4 * qi
                    if kd >= 0:  # block straddles the diagonal
                        nc.vector.tensor_mul(pt[:], pt[:], mask_sb[:, kd, :])
                    for qs in range(4):
                        if jt <= 4 * qi + qs:
                            nc.tensor.matmul(
                                oacc[qs][:],
                                lhsT=pt[:, bass.ts(qs, 128)],
                                rhs=vaug[:, jt, :],
                                start=(jt == 0),
                                stop=(jt == 4 * qi + qs),
                            )
                for qs in range(4):
                    rec = spool.tile([128, 1], F32, tag="rec")
                    nc.vector.reciprocal(rec[:], oacc[qs][:, 128:129])
                    onrm = spool.tile([128, 128], BF16, tag="onrm")
                    nc.scalar.activation(onrm[:], oacc[qs][:, 0:128], Copy, scale=rec[:])
                    tp = tpsum.tile([128, 128], BF16, tag="tp")
                    nc.tensor.transpose(tp[:], onrm[:], ident[:])
                    nc.vector.tensor_copy(
                        oT[:, h, bass.ds(qi * 512 + qs * 128, 128)], tp[:]
                    )

        # Phase 5: partial o_proj, y_c = O^T.T @ Wo_c.
        for ti in range(NT):
            yr = yrow.tile([128, D], F32, tag="yr")
            for mi in range(NM):
                yp = psum.tile([128, 512], F32, tag="ps")
                for h2 in range(HL):
                    nc.tensor.matmul(
                        yp[:],
                        lhsT=oT[:, h2, bass.ts(ti, 128)],
                        rhs=wo_sb[:, h2, bass.ts(mi, 512)],
                        start=(h2 == 0),
                        stop=(h2 == HL - 1),
                    )
                nc.vector.tensor_copy(yr[:, bass.ts(mi, 512)], yp[:])
            nc.sync.dma_start(y[bass.ts(ti, 128), :], yr[:])


_CACHE = {}


def _get_program():
    if "nc" not in _CACHE:
        nc = bacc.Bacc(
            "TRN2", target_bir_lowering=False, debug=False, num_devices=N_CORES
        )
        _build(nc)
        nc.compile()
        _CACHE["nc"] = nc
    return _CACHE["nc"]


def _rope_tables():
    inv_freq = 1.0 / (ROPE_BASE ** (np.arange(64, dtype=np.float64) / 64))
    ang = np.arange(T, dtype=np.float64)[:, None] * inv_freq[None, :]  # [T, 64]
    cos = np.cos(ang).T  # [64, T]
    sin = np.sin(ang).T
    cos2 = np.concatenate([cos, cos], axis=0).astype(NPBF16)
    sinsig = np.concatenate([-sin, sin], axis=0).astype(NPBF16)
    return cos2, sinsig


def _diag_masks():
    p = np.arange(128)[:, None, None]
    kd = np.arange(4)[None, :, None]
    f = np.arange(512)[None, None, :]
    return (f >= kd * 128 + p).astype(NPBF16)


def kernel(x, Wq, Wk, Wv, Wo):
    x = np.asarray(x, dtype=np.float32)
    Wq = np.asarray(Wq, dtype=np.float32)
    Wk = np.asarray(Wk, dtype=np.float32)
    Wv = np.asarray(Wv, dtype=np.float32)
    Wo = np.asarray(Wo, dtype=np.float32)

    # x[t, c] -> xp[n, p, k, j] = x[n*512+j, k*128+p]; contiguous per partition.
    xp = np.ascontiguousarray(
        x.reshape(T, D).reshape(NQ, 512, KC, 128).transpose(0, 3, 2, 1)
    ).astype(NPBF16)
    cos2, sinsig = _rope_tables()
    maskd = _diag_masks()
    d_idx = np.arange(128)
    permm = (d_idx[:, None] == (d_idx[None, :] + 64) % 128).astype(NPBF16)

    in_maps = []
    for c in range(N_CORES):
        h0, h1 = 2 * c, 2 * c + 1
        kv = c // 2
        wqkv_c = np.concatenate(
            [
                Wq[:, h0 * DH:(h0 + 1) * DH],
                Wq[:, h1 * DH:(h1 + 1) * DH],
                Wk[:, kv * DH:(kv + 1) * DH],
                Wv[:, kv * DH:(kv + 1) * DH],
            ],
            axis=1,
        )  # [D, 512]
        wqkv_pre = np.ascontiguousarray(
            wqkv_c.reshape(KC, 128, 4, 128).transpose(1, 0, 2, 3)
        ).astype(NPBF16)
        wo_pre = np.ascontiguousarray(
            np.stack(
                [Wo[h0 * DH:(h0 + 1) * DH, :], Wo[h1 * DH:(h1 + 1) * DH, :]], axis=0
            ).transpose(1, 0, 2)
        ).astype(NPBF16)
        in_maps.append(
            {
                "xp": xp,
                "wqkv": wqkv_pre,
                "wo": wo_pre,
                "cos2": cos2,
                "sinsig": sinsig,
                "perm": permm,
                "maskd": maskd,
            }
        )

    nc = _get_program()
    res = run_bass_kernel_spmd(nc, in_maps, list(range(N_CORES)))
    out = np.zeros((T, D), dtype=np.float32)
    for c in range(N_CORES):
        out += res.results[c]["y"]
    return out.reshape(B, T, D)
